# revision 36
# baseline (speedup 1.0000x reference)
import sys

sys.path.insert(0, "/opt/trn_rl_repo")
import numpy as np
import concourse.bacc as bacc
import concourse.mybir as mybir
import concourse.tile as tile
from concourse.bass_utils import run_bass_kernel_spmd

F32R = mybir.dt.float32r
F32 = mybir.dt.float32
FP16 = mybir.dt.float16
AF = mybir.ActivationFunctionType

B, S, D, H, DV = 2, 2048, 1024, 16, 64
NKT = 8     # k-tiles of 128 over D
NJ = 4      # query chunks of 512
NB = 16     # key blocks of 128
HPC = 4     # heads per core

_NC = None


def _build(debug=False):
    nc = bacc.Bacc(target_bir_lowering=False)
    xq = nc.dram_tensor("xq", [D, S], FP16, kind="ExternalInput")
    xk = nc.dram_tensor("xk", [D, S], FP16, kind="ExternalInput")
    xv = nc.dram_tensor("xv", [D, S], FP16, kind="ExternalInput")
    wq = nc.dram_tensor("wq", [D, 256], FP16, kind="ExternalInput")
    wk = nc.dram_tensor("wk", [D, 256], FP16, kind="ExternalInput")
    wv = nc.dram_tensor("wv", [D, 256], FP16, kind="ExternalInput")
    w0 = nc.dram_tensor("w0", [256, D], FP16, kind="ExternalInput")
    cmt = nc.dram_tensor("cmt", [128, 512], F32R, kind="ExternalInput")
    yt = nc.dram_tensor("yt", [D, S], FP16, kind="ExternalOutput")
    if debug:
        qt_d = nc.dram_tensor("qt_d", [4, 128, S], FP16, kind="ExternalOutput")
        kt_d = nc.dram_tensor("kt_d", [2, 128, S], FP16, kind="ExternalOutput")
        v_d = nc.dram_tensor("v_d", [128, NB, HPC, 128], F32R, kind="ExternalOutput")
        ot_d = nc.dram_tensor("ot_d", [2, 128, S], FP16, kind="ExternalOutput")
        st_d = nc.dram_tensor("st_d", [128, 1024], F32R, kind="ExternalOutput")
        pt_d = nc.dram_tensor("pt_d", [128, 1024], F32R, kind="ExternalOutput")
        op_d = nc.dram_tensor("op_d", [128, 512], F32R, kind="ExternalOutput")

    with tile.TileContext(nc) as tc:
        with tc.tile_pool(name="pp", bufs=1) as pp:
            # Per-head Q with the other head's 64 rows zeroed: scores can then
            # use the full dense 128-row kt block as stationary (full PE
            # array) -- the zero rows kill the other head's contribution.
            qtz = [pp.tile([128, S], FP16, name=f"qtz{h}", tag=f"qtz{h}")
                   for h in range(HPC)]
            kt = [pp.tile([128, S], FP16, name=f"kt{p}", tag=f"kt{p}") for p in range(2)]
            # V padded to 128 cols, all non-V columns = ones. Even heads keep
            # V in cols 0:64 (numerators -> out rows 0:64, den at row 64);
            # odd heads keep V in cols 64:128 (numerators -> out rows 64:128,
            # den read from row 32). This keeps every normalize step
            # partition-aligned and lets the out-projection consume a
            # pair-stacked [128, S] activation with a full 128-row stationary.
            v2 = pp.tile([128, NB, HPC, 128], F32R, name="v2", tag="v2")
            # bcsel rows 0 and 64: all-ones [1,128] stationaries that
            # broadcast the den row (even heads: opsum row 64; odd heads:
            # opsum row 0) onto all 128 partitions of bcps.
            bcsel = pp.tile([128, 128], F32R, name="bcsel", tag="bcsel")
            w0p = [pp.tile([128, D], FP16, name=f"w0p{p}", tag=f"w0p{p}") for p in range(2)]
            ot2 = [pp.tile([128, S], FP16, name=f"ot2{p}", tag=f"ot2{p}") for p in range(2)]
            cm_sb = pp.tile([128, 512], F32R, name="cmsb", tag="cmsb")

            # ones / zero init (stays valid for the whole kernel)
            ones_stage = pp.tile([128, 512], F32, name="ones_stage", tag="ones_stage")
            nc.vector.memset(ones_stage[:, :], 1.0)
            for i in range(NB):
                nc.vector.tensor_copy(v2[:, i, :, :], ones_stage[:, :])
            nc.vector.tensor_copy(bcsel[64:65, :], ones_stage[64:65, 0:128])
            nc.vector.tensor_copy(bcsel[0:1, :], ones_stage[0:1, 0:128])
            for h in range(HPC):
                dead = 64 * (1 - (h % 2))
                nc.vector.memset(qtz[h][dead:dead + 64, :], 0.0)

            # ---- Phase A: projections ----
            with tc.tile_pool(name="wts", bufs=1) as wts, \
                 tc.tile_pool(name="xin", bufs=1) as xin, \
                 tc.tile_pool(name="psA", bufs=8, space="PSUM") as psA:
                # DMA order: each weight tensor lands just before the x
                # tiles that feed its projection; x tiles round-robin across
                # the two hw DMA queues in PE consumption order.
                wv_t, wq_t, wk_t = [], [], []
                for lst, dram, tag in ((wv_t, wv, "wv"), (wq_t, wq, "wq"),
                                       (wk_t, wk, "wk")):
                    for k in range(NKT):
                        t = wts.tile([128, 256], FP16, name=f"{tag}{k}", tag=f"{tag}{k}")
                        lst.append(t)
                xv_t, xq_t, xk_t = [], [], []
                for lst, dram, tag in ((xv_t, xv, "xv"), (xq_t, xq, "xq"),
                                       (xk_t, xk, "xk")):
                    for k in range(NKT):
                        t = xin.tile([128, S], FP16, name=f"{tag}{k}", tag=tag,
                                     bufs=8)
                        lst.append(t)
                for wlst, wdram, xlst, xdram in ((wv_t, wv, xv_t, xv),
                                                 (wq_t, wq, xq_t, xq),
                                                 (wk_t, wk, xk_t, xk)):
                    for k in range(NKT):
                        nc.scalar.dma_start(out=wlst[k][:, :],
                                            in_=wdram[128 * k:128 * k + 128, :])
                    for k in range(NKT):
                        eng = nc.sync if k % 2 == 0 else nc.scalar
                        eng.dma_start(out=xlst[k][:, :],
                                      in_=xdram[128 * k:128 * k + 128, :])
                for p in range(2):
                    nc.sync.dma_start(out=w0p[p][:, :], in_=w0[128 * p:128 * p + 128, :])
                nc.sync.dma_start(out=cm_sb[:, :], in_=cmt[:, :])

                # V projection: 2 waves x 8 st-groups, kt-outer within a wave
                for w in range(2):
                    vps = [psA.tile([128, 4, 64], F32, name=f"vps{w}{g}", tag="pj",
                                    bufs=8)
                           for g in range(8)]
                    for k in range(NKT):
                        for g in range(8):
                            st = 8 * w + g
                            nc.tensor.matmul(
                                vps[g][:, :, :],
                                xv_t[k][:, 128 * st:128 * st + 128],
                                wv_t[k][:, :],
                                start=(k == 0), stop=(k == NKT - 1))
                    for g in range(8):
                        nc.vector.tensor_copy(v2[:, 8 * w + g, 0:4:2, 0:64],
                                              vps[g][:, 0:4:2, :])
                        nc.vector.tensor_copy(v2[:, 8 * w + g, 1:4:2, 64:128],
                                              vps[g][:, 1:4:2, :])

                # Q/K projections: N=256 matmuls, stationary stable across 8 cols
                for which, wt, xt in (("q", wq_t, xq_t), ("k", wk_t, xk_t)):
                    qps = [psA.tile([128, 512], F32, name=f"pj{i}", tag="pj", bufs=8)
                           for i in range(8)]
                    for k in range(NKT):
                        for p in range(2):
                            for c in range(8):
                                jj, half = c // 2, c % 2
                                nc.tensor.matmul(
                                    qps[4 * p + jj][:, 256 * half:256 * half + 256],
                                    wt[k][:, 128 * p:128 * p + 128],
                                    xt[k][:, 256 * c:256 * c + 256],
                                    start=(k == 0 and half == 0),
                                    stop=(k == NKT - 1 and half == 1))
                    for p in range(2):
                        for jj in range(4):
                            sl = slice(512 * jj, 512 * jj + 512)
                            if which == "q":
                                nc.vector.tensor_copy(qtz[2 * p][0:64, sl],
                                                      qps[4 * p + jj][0:64, :])
                                nc.vector.tensor_copy(qtz[2 * p + 1][64:128, sl],
                                                      qps[4 * p + jj][64:128, :])
                            else:
                                nc.vector.tensor_copy(kt[p][:, sl],
                                                      qps[4 * p + jj][:, :])

            # ---- Phase B/C: flat software-pipelined attention + out-proj ----
            with tc.tile_pool(name="pb", bufs=1) as pb, \
                 tc.tile_pool(name="psB", bufs=1, space="PSUM") as psB:

                def emit_phase_c_group(jp, e):
                    yps = psB.tile([128, 512], F32, name="yps", tag="yps", bufs=2)
                    for p in range(2):
                        for half in range(2):
                            nc.tensor.matmul(
                                yps[:, 256 * half:256 * half + 256],
                                w0p[p][:, 128 * e:128 * e + 128],
                                ot2[p][:, 512 * jp + 256 * half:512 * jp + 256 * half + 256],
                                start=(p == 0 and half == 0),
                                stop=(p == 1 and half == 1))
                    ysb = pb.tile([128, 512], FP16, name="ysb", tag="ysb", bufs=3)
                    nc.vector.tensor_copy(ysb[:, :], yps[:, :])
                    nc.sync.dma_start(out=yt[128 * e:128 * e + 128, 512 * jp:512 * jp + 512],
                                      in_=ysb[:, :])

                # HAM warm-up burst: dependency-free full-array matmuls to
                # carry K=8/8 across the phase A->B transition bubble.
                wps = psB.tile([128, 512], F32, name="wps", tag="yps", bufs=2)
                for wi in range(24):
                    nc.tensor.matmul(
                        wps[:, 256 * (wi % 2):256 * (wi % 2) + 256],
                        qtz[wi % 4][:, 0:128],
                        qtz[(wi + 1) % 4][:, 0:256],
                        start=(wi < 2), stop=(wi >= 22))

                for j in range(NJ):
                    trips = []
                    for h in range(HPC):
                        for t in range(2 * j):
                            trips.append(("full", h, [2 * t, 2 * t + 1]))
                        trips.append(("diagA", h, None))
                        trips.append(("diagB", h, None))
                    G = len(trips)

                    def sc_mm(stile, col, width, blk, qoff, pair, h, first, last):
                        nc.tensor.matmul(
                            stile[:, col:col + width],
                            kt[pair][:, 128 * blk:128 * blk + 128],
                            qtz[h][:, qoff:qoff + width],
                            start=first, stop=last)

                    def emit_scores(tr):
                        kind, h, blks = tr
                        pair = h // 2
                        qb = 512 * j
                        stile = psB.tile([128, 1024], F32, name="stile", tag="stile", bufs=2)
                        if kind == "full":
                            for n, blk in enumerate(blks):
                                for half in range(2):
                                    sc_mm(stile, 512 * n + 256 * half, 256, blk,
                                          qb + 256 * half, pair, h,
                                          half == 0, half == 1)
                        elif kind == "diagA":
                            for dd in range(4):
                                sc_mm(stile, 128 * dd, 128, 4 * j + dd,
                                      qb + 128 * dd, pair, h, dd == 0, dd == 3)
                            sc_mm(stile, 512, 256, 4 * j, qb + 128, pair, h, True, False)
                            sc_mm(stile, 768, 128, 4 * j, qb + 384, pair, h, False, True)
                        else:  # diagB
                            sc_mm(stile, 0, 256, 4 * j + 1, qb + 256, pair, h, True, False)
                            sc_mm(stile, 256, 128, 4 * j + 2, qb + 384, pair, h, False, True)
                        return stile

                    def emit_exp(tr, stile):
                        kind, h, blks = tr
                        w = {"full": 1024, "diagA": 896, "diagB": 384}[kind]
                        ptt = pb.tile([128, 1024], F32R, name="ptt", tag="ptt", bufs=4)
                        nc.scalar.activation(ptt[:, 0:w], stile[:, 0:w], AF.Exp)
                        if kind == "diagA":
                            nc.vector.tensor_mul(ptt[:, 0:512], ptt[:, 0:512], cm_sb[:, :])
                        return ptt

                    def emit_pv(tr, ptt, opsum, first):
                        kind, h, blks = tr

                        def pv(col, width, blk, pcol, fb, lb=False):
                            nc.tensor.matmul(
                                opsum[:, col:col + width],
                                v2[:, blk, h, :],
                                ptt[:, pcol:pcol + width],
                                start=fb, stop=lb)

                        if kind == "full":
                            for n, blk in enumerate(blks):
                                for half in range(2):
                                    pv(256 * half, 256, blk, 512 * n + 256 * half,
                                       first and n == 0 and half == 0)
                        elif kind == "diagA":
                            for dd in range(4):
                                pv(128 * dd, 128, 4 * j + dd, 128 * dd, first and dd == 0)
                            pv(128, 256, 4 * j, 512, False)
                            pv(384, 128, 4 * j, 768, False)
                        else:
                            pv(256, 256, 4 * j + 1, 0, False)
                            pv(384, 128, 4 * j + 2, 256, False, lb=True)

                    def emit_norm(h, opsum):
                        # den row -> SBUF, PE-broadcast onto the head's own
                        # partition half, reciprocal, normalize.  Even heads:
                        # num rows 0:64, den row 64.  Odd heads: num rows
                        # 64:128, den row 0 (every ones column repeats den).
                        drow = 64 if h % 2 == 0 else 0
                        obase = 0 if h % 2 == 0 else 64
                        pair = h // 2
                        den = pb.tile([128, 512], F32R, name="den", tag="den", bufs=3)
                        nc.vector.tensor_copy(den[drow:drow + 1, :],
                                              opsum[drow:drow + 1, :])
                        bcps = psB.tile([128, 512], F32, name="bcps", tag="yps",
                                        bufs=2)
                        for half in range(2):
                            nc.tensor.matmul(
                                bcps[:, 256 * half:256 * half + 256],
                                bcsel[drow:drow + 1, :],
                                den[drow:drow + 1, 256 * half:256 * half + 256],
                                start=(half == 0), stop=(half == 1))
                        rec = pb.tile([128, 512], F32, name="rec", tag="rec", bufs=3)
                        nc.vector.reciprocal_approx_fast(rec[:, :], bcps[:, :])
                        nc.vector.tensor_mul(
                            ot2[pair][obase:obase + 64, 512 * j:512 * j + 512],
                            opsum[obase:obase + 64, :],
                            rec[obase:obase + 64, :])

                    stiles, ptts, opsums = {}, {}, {}
                    pc_queue = list(range(8)) if j > 0 else []
                    for g in range(G + 2):
                        # PE filler with no ACT dependency first
                        if pc_queue and g >= 6:
                            emit_phase_c_group(j - 1, pc_queue.pop(0))
                        if 2 <= g:
                            tr = trips[g - 2]
                            h = tr[1]
                            is_first_trip_of_h = (tr[0] == "diagA" and j == 0) or \
                                (tr[0] == "full" and tr[2][0] == 0)
                            if is_first_trip_of_h:
                                opsums[h] = psB.tile([128, 512], F32, name="opsum",
                                                     tag="acc", bufs=2)
                            emit_pv(tr, ptts.pop(g - 2), opsums[h], is_first_trip_of_h)
                            if tr[0] == "diagB":
                                op_done = opsums.pop(h)
                                if debug and j == 0 and h == 0:
                                    dbg_op_src = op_done
                                emit_norm(h, op_done)
                        if g < G:
                            stiles[g] = emit_scores(trips[g])
                            if debug and j == 0 and g == 0:
                                dbg_sb = pb.tile([128, 1024], F32R, name="dbgsb",
                                                 tag="dbgsb")
                                nc.vector.tensor_copy(dbg_sb[:, :], stiles[g][:, :])
                                nc.sync.dma_start(out=st_d[:, :], in_=dbg_sb[:, :])
                        if 1 <= g <= G:
                            ptts[g - 1] = emit_exp(trips[g - 1], stiles.pop(g - 1))
                            if debug and j == 0 and g == 1:
                                nc.sync.dma_start(out=pt_d[:, :], in_=ptts[g - 1][:, :])
                            if debug and j == 0 and g == 4:
                                dbg_op = pb.tile([128, 512], F32R, name="dbgop",
                                                 tag="dbgop")
                                nc.vector.tensor_copy(dbg_op[:, :], dbg_op_src[:, :])
                                nc.sync.dma_start(out=op_d[:, :], in_=dbg_op[:, :])
                    # drain any leftover phase C groups of j-1
                    while pc_queue:
                        emit_phase_c_group(j - 1, pc_queue.pop(0))

                # final out-proj for j=3
                for e in range(8):
                    emit_phase_c_group(NJ - 1, e)

                if debug:
                    for h in range(HPC):
                        nc.sync.dma_start(out=qt_d[h, :, :], in_=qtz[h][:, :])
                    for p in range(2):
                        nc.sync.dma_start(out=kt_d[p, :, :], in_=kt[p][:, :])
                        nc.sync.dma_start(out=ot_d[p, :, :], in_=ot2[p][:, :])
                    nc.sync.dma_start(out=v_d[:, :, :, :], in_=v2[:, :, :, :])

    nc.compile()
    return nc


def _run(inputs, trace=False, debug=False):
    global _NC
    if _NC is None:
        _NC = _build(debug=debug)
    q = np.asarray(inputs["q"], dtype=np.float32)
    k = np.asarray(inputs["k"], dtype=np.float32)
    v = np.asarray(inputs["v"], dtype=np.float32)
    w_query = np.asarray(inputs["w_query"], dtype=np.float32)
    w_key = np.asarray(inputs["w_key"], dtype=np.float32)
    w_value = np.asarray(inputs["w_value"], dtype=np.float32)
    w_0 = np.asarray(inputs["w_0"], dtype=np.float32)

    tri = np.triu(np.ones((128, 128), dtype=np.float32))
    cmt = np.ascontiguousarray(np.tile(tri, (1, 4)))

    xq_b = [np.ascontiguousarray(q[b].T).astype(np.float16) for b in range(B)]
    xk_b = [np.ascontiguousarray(k[b].T).astype(np.float16) for b in range(B)]
    xv_b = [np.ascontiguousarray(v[b].T).astype(np.float16) for b in range(B)]

    in_maps = []
    for c in range(8):
        b, g = c // 4, c % 4
        sl = slice(256 * g, 256 * g + 256)
        in_maps.append({
            "xq": xq_b[b], "xk": xk_b[b], "xv": xv_b[b],
            "wq": np.ascontiguousarray(w_query[sl, :].T).astype(np.float16),
            "wk": np.ascontiguousarray(w_key[sl, :].T).astype(np.float16),
            "wv": np.ascontiguousarray(w_value[sl, :].T).astype(np.float16),
            "w0": np.ascontiguousarray(w_0[:, sl].T).astype(np.float16),
            "cmt": cmt,
        })

    res = run_bass_kernel_spmd(_NC, in_maps, core_ids=list(range(8)), trace=trace)
    y = np.empty((B, S, D), dtype=np.float32)
    for b in range(B):
        acc = res.results[4 * b]["yt"].astype(np.float32)
        for g in range(1, 4):
            acc += res.results[4 * b + g]["yt"].astype(np.float32)
        y[b] = acc.T
    if debug:
        return y, getattr(res, "exec_time_ns", None), res
    return y, getattr(res, "exec_time_ns", None)


def kernel(**inputs):
    return _run(inputs, trace=False)[0]


# revision 38
# speedup vs baseline: 1.0397x; 1.0397x over previous
import sys

sys.path.insert(0, "/opt/trn_rl_repo")
import numpy as np
import concourse.bacc as bacc
import concourse.mybir as mybir
import concourse.tile as tile
from concourse.bass_utils import run_bass_kernel_spmd

F32R = mybir.dt.float32r
F32 = mybir.dt.float32
FP16 = mybir.dt.float16
AF = mybir.ActivationFunctionType

B, S, D, H, DV = 2, 2048, 1024, 16, 64
NKT = 8     # k-tiles of 128 over D
NJ = 4      # query chunks of 512
NB = 16     # key blocks of 128
HPC = 4     # heads per core

_NC = None


def _build(debug=False):
    nc = bacc.Bacc(target_bir_lowering=False)
    xq = nc.dram_tensor("xq", [D, S], FP16, kind="ExternalInput")
    xk = nc.dram_tensor("xk", [D, S], FP16, kind="ExternalInput")
    xv = nc.dram_tensor("xv", [D, S], FP16, kind="ExternalInput")
    wq = nc.dram_tensor("wq", [D, 256], FP16, kind="ExternalInput")
    wk = nc.dram_tensor("wk", [D, 256], FP16, kind="ExternalInput")
    wv = nc.dram_tensor("wv", [D, 256], FP16, kind="ExternalInput")
    w0 = nc.dram_tensor("w0", [256, D], FP16, kind="ExternalInput")
    cmt = nc.dram_tensor("cmt", [128, 512], F32R, kind="ExternalInput")
    yt = nc.dram_tensor("yt", [D, S], FP16, kind="ExternalOutput")
    if debug:
        qt_d = nc.dram_tensor("qt_d", [4, 128, S], FP16, kind="ExternalOutput")
        kt_d = nc.dram_tensor("kt_d", [2, 128, S], FP16, kind="ExternalOutput")
        v_d = nc.dram_tensor("v_d", [128, NB, HPC, 128], F32R, kind="ExternalOutput")
        ot_d = nc.dram_tensor("ot_d", [2, 128, S], FP16, kind="ExternalOutput")
        st_d = nc.dram_tensor("st_d", [128, 1024], F32R, kind="ExternalOutput")
        pt_d = nc.dram_tensor("pt_d", [128, 1024], F32R, kind="ExternalOutput")
        op_d = nc.dram_tensor("op_d", [128, 512], F32R, kind="ExternalOutput")

    with tile.TileContext(nc) as tc:
        with tc.tile_pool(name="pp", bufs=1) as pp:
            # Per-head Q with the other head's 64 rows zeroed: scores can then
            # use the full dense 128-row kt block as stationary (full PE
            # array) -- the zero rows kill the other head's contribution.
            qtz = [pp.tile([128, S], FP16, name=f"qtz{h}", tag=f"qtz{h}")
                   for h in range(HPC)]
            kt = [pp.tile([128, S], FP16, name=f"kt{p}", tag=f"kt{p}") for p in range(2)]
            # V padded to 128 cols, all non-V columns = ones. Even heads keep
            # V in cols 0:64 (numerators -> out rows 0:64, den at row 64);
            # odd heads keep V in cols 64:128 (numerators -> out rows 64:128,
            # den read from row 32). This keeps every normalize step
            # partition-aligned and lets the out-projection consume a
            # pair-stacked [128, S] activation with a full 128-row stationary.
            v2 = pp.tile([128, NB, HPC, 128], F32R, name="v2", tag="v2")
            # bcsel rows 0 and 64: all-ones [1,128] stationaries that
            # broadcast the den row (even heads: opsum row 64; odd heads:
            # opsum row 0) onto all 128 partitions of bcps.
            bcsel = pp.tile([128, 128], F32R, name="bcsel", tag="bcsel")
            w0p = [pp.tile([128, D], FP16, name=f"w0p{p}", tag=f"w0p{p}") for p in range(2)]
            ot2 = [pp.tile([128, S], FP16, name=f"ot2{p}", tag=f"ot2{p}") for p in range(2)]
            cm_sb = pp.tile([128, 512], F32R, name="cmsb", tag="cmsb")

            # ones / zero init (stays valid for the whole kernel)
            ones_stage = pp.tile([128, 512], F32, name="ones_stage", tag="ones_stage")
            nc.vector.memset(ones_stage[:, :], 1.0)
            for i in range(NB):
                nc.vector.tensor_copy(v2[:, i, :, :], ones_stage[:, :])
            nc.vector.tensor_copy(bcsel[64:65, :], ones_stage[64:65, 0:128])
            nc.vector.tensor_copy(bcsel[0:1, :], ones_stage[0:1, 0:128])
            for h in range(HPC):
                dead = 64 * (1 - (h % 2))
                nc.vector.memset(qtz[h][dead:dead + 64, :], 0.0)

            # ---- Phase A: projections ----
            with tc.tile_pool(name="wts", bufs=1) as wts, \
                 tc.tile_pool(name="xin", bufs=1) as xin, \
                 tc.tile_pool(name="psA", bufs=8, space="PSUM") as psA:
                # DMA order: each weight tensor lands just before the x
                # tiles that feed its projection; x tiles round-robin across
                # the two hw DMA queues in PE consumption order.
                wv_t, wq_t, wk_t = [], [], []
                for lst, dram, tag in ((wv_t, wv, "wv"), (wq_t, wq, "wq"),
                                       (wk_t, wk, "wk")):
                    for k in range(NKT):
                        t = wts.tile([128, 256], FP16, name=f"{tag}{k}", tag=f"{tag}{k}")
                        lst.append(t)
                xv_t, xq_t, xk_t = [], [], []
                for lst, dram, tag in ((xv_t, xv, "xv"), (xq_t, xq, "xq"),
                                       (xk_t, xk, "xk")):
                    for k in range(NKT):
                        t = xin.tile([128, S], FP16, name=f"{tag}{k}", tag=tag,
                                     bufs=8)
                        lst.append(t)
                for wlst, wdram, xlst, xdram in ((wv_t, wv, xv_t, xv),
                                                 (wq_t, wq, xq_t, xq),
                                                 (wk_t, wk, xk_t, xk)):
                    for k in range(NKT):
                        nc.scalar.dma_start(out=wlst[k][:, :],
                                            in_=wdram[128 * k:128 * k + 128, :])
                    for k in range(NKT):
                        eng = nc.sync if k % 2 == 0 else nc.scalar
                        eng.dma_start(out=xlst[k][:, :],
                                      in_=xdram[128 * k:128 * k + 128, :])
                for p in range(2):
                    nc.sync.dma_start(out=w0p[p][:, :], in_=w0[128 * p:128 * p + 128, :])
                nc.sync.dma_start(out=cm_sb[:, :], in_=cmt[:, :])

                # V projection: 2 waves x 8 st-groups, kt-outer within a wave
                for w in range(2):
                    vps = [psA.tile([128, 4, 64], F32, name=f"vps{w}{g}", tag="pj",
                                    bufs=8)
                           for g in range(8)]
                    for k in range(NKT):
                        for g in range(8):
                            st = 8 * w + g
                            nc.tensor.matmul(
                                vps[g][:, :, :],
                                xv_t[k][:, 128 * st:128 * st + 128],
                                wv_t[k][:, :],
                                start=(k == 0), stop=(k == NKT - 1))
                    for g in range(8):
                        nc.vector.tensor_copy(v2[:, 8 * w + g, 0:4:2, 0:64],
                                              vps[g][:, 0:4:2, :])
                        nc.vector.tensor_copy(v2[:, 8 * w + g, 1:4:2, 64:128],
                                              vps[g][:, 1:4:2, :])

                # Q/K projections: N=256 matmuls, stationary stable across 8 cols
                for which, wt, xt in (("q", wq_t, xq_t), ("k", wk_t, xk_t)):
                    qps = [psA.tile([128, 512], F32, name=f"pj{i}", tag="pj", bufs=8)
                           for i in range(8)]
                    for k in range(NKT):
                        for p in range(2):
                            for c in range(8):
                                jj, half = c // 2, c % 2
                                nc.tensor.matmul(
                                    qps[4 * p + jj][:, 256 * half:256 * half + 256],
                                    wt[k][:, 128 * p:128 * p + 128],
                                    xt[k][:, 256 * c:256 * c + 256],
                                    start=(k == 0 and half == 0),
                                    stop=(k == NKT - 1 and half == 1))
                    for p in range(2):
                        for jj in range(4):
                            sl = slice(512 * jj, 512 * jj + 512)
                            if which == "q":
                                nc.vector.tensor_copy(qtz[2 * p][0:64, sl],
                                                      qps[4 * p + jj][0:64, :])
                                nc.vector.tensor_copy(qtz[2 * p + 1][64:128, sl],
                                                      qps[4 * p + jj][64:128, :])
                            else:
                                nc.vector.tensor_copy(kt[p][:, sl],
                                                      qps[4 * p + jj][:, :])

            # ---- Phase B/C: flat software-pipelined attention + out-proj ----
            with tc.tile_pool(name="pb", bufs=1) as pb, \
                 tc.tile_pool(name="psB", bufs=1, space="PSUM") as psB:

                def emit_phase_c_group(jp, e):
                    yps = psB.tile([128, 512], F32, name="yps", tag="yps", bufs=2)
                    for p in range(2):
                        for half in range(2):
                            nc.tensor.matmul(
                                yps[:, 256 * half:256 * half + 256],
                                w0p[p][:, 128 * e:128 * e + 128],
                                ot2[p][:, 512 * jp + 256 * half:512 * jp + 256 * half + 256],
                                start=(p == 0 and half == 0),
                                stop=(p == 1 and half == 1))
                    ysb = pb.tile([128, 512], FP16, name="ysb", tag="ysb", bufs=3)
                    nc.vector.tensor_copy(ysb[:, :], yps[:, :])
                    nc.sync.dma_start(out=yt[128 * e:128 * e + 128, 512 * jp:512 * jp + 512],
                                      in_=ysb[:, :])

                # HAM warm-up burst: dependency-free full-array matmuls to
                # carry K=8/8 across the phase A->B transition bubble.
                wps = psB.tile([128, 512], F32, name="wps", tag="yps", bufs=2)
                for wi in range(24):
                    nc.tensor.matmul(
                        wps[:, 256 * (wi % 2):256 * (wi % 2) + 256],
                        qtz[wi % 4][:, 0:128],
                        qtz[(wi + 1) % 4][:, 0:256],
                        start=(wi < 2), stop=(wi >= 22))

                for j in range(NJ):
                    trips = []
                    for h in range(HPC):
                        for t in range(2 * j):
                            trips.append(("full", h, [2 * t, 2 * t + 1]))
                        trips.append(("diagA", h, None))
                        trips.append(("diagB", h, None))
                    G = len(trips)

                    def sc_mm(stile, col, width, blk, qoff, pair, h, first, last):
                        nc.tensor.matmul(
                            stile[:, col:col + width],
                            kt[pair][:, 128 * blk:128 * blk + 128],
                            qtz[h][:, qoff:qoff + width],
                            start=first, stop=last)

                    def emit_scores(tr):
                        kind, h, blks = tr
                        pair = h // 2
                        qb = 512 * j
                        stile = psB.tile([128, 1024], F32, name="stile", tag="stile", bufs=2)
                        if kind == "full":
                            for n, blk in enumerate(blks):
                                for half in range(2):
                                    sc_mm(stile, 512 * n + 256 * half, 256, blk,
                                          qb + 256 * half, pair, h,
                                          half == 0, half == 1)
                        elif kind == "diagA":
                            for dd in range(4):
                                sc_mm(stile, 128 * dd, 128, 4 * j + dd,
                                      qb + 128 * dd, pair, h, dd == 0, dd == 3)
                            sc_mm(stile, 512, 256, 4 * j, qb + 128, pair, h, True, False)
                            sc_mm(stile, 768, 128, 4 * j, qb + 384, pair, h, False, True)
                        else:  # diagB
                            sc_mm(stile, 0, 256, 4 * j + 1, qb + 256, pair, h, True, False)
                            sc_mm(stile, 256, 128, 4 * j + 2, qb + 384, pair, h, False, True)
                        return stile

                    def emit_exp(tr, stile):
                        kind, h, blks = tr
                        w = {"full": 1024, "diagA": 896, "diagB": 384}[kind]
                        ptt = pb.tile([128, 1024], F32R, name="ptt", tag="ptt", bufs=3)
                        nc.scalar.activation(ptt[:, 0:w], stile[:, 0:w], AF.Exp)
                        if kind == "diagA":
                            nc.vector.tensor_mul(ptt[:, 0:512], ptt[:, 0:512], cm_sb[:, :])
                        return ptt

                    def emit_pv(tr, ptt, opsum, first):
                        kind, h, blks = tr

                        def pv(col, width, blk, pcol, fb, lb=False):
                            nc.tensor.matmul(
                                opsum[:, col:col + width],
                                v2[:, blk, h, :],
                                ptt[:, pcol:pcol + width],
                                start=fb, stop=lb)

                        if kind == "full":
                            for n, blk in enumerate(blks):
                                for half in range(2):
                                    pv(256 * half, 256, blk, 512 * n + 256 * half,
                                       first and n == 0 and half == 0)
                        elif kind == "diagA":
                            for dd in range(4):
                                pv(128 * dd, 128, 4 * j + dd, 128 * dd, first and dd == 0)
                            pv(128, 256, 4 * j, 512, False)
                            pv(384, 128, 4 * j, 768, False)
                        else:
                            pv(256, 256, 4 * j + 1, 0, False)
                            pv(384, 128, 4 * j + 2, 256, False, lb=True)

                    def emit_norm(h, opsum):
                        # den row -> SBUF, PE-broadcast onto the head's own
                        # partition half, reciprocal, normalize.  Even heads:
                        # num rows 0:64, den row 64.  Odd heads: num rows
                        # 64:128, den row 0 (every ones column repeats den).
                        drow = 64 if h % 2 == 0 else 0
                        obase = 0 if h % 2 == 0 else 64
                        pair = h // 2
                        den = pb.tile([128, 512], F32R, name="den", tag="den", bufs=2)
                        nc.vector.tensor_copy(den[drow:drow + 1, :],
                                              opsum[drow:drow + 1, :])
                        bcps = psB.tile([128, 512], F32, name="bcps", tag="yps",
                                        bufs=2)
                        for half in range(2):
                            nc.tensor.matmul(
                                bcps[:, 256 * half:256 * half + 256],
                                bcsel[drow:drow + 1, :],
                                den[drow:drow + 1, 256 * half:256 * half + 256],
                                start=(half == 0), stop=(half == 1))
                        rec = pb.tile([128, 512], F32, name="rec", tag="rec", bufs=2)
                        nc.vector.reciprocal_approx_fast(rec[:, :], bcps[:, :])
                        nc.vector.tensor_mul(
                            ot2[pair][obase:obase + 64, 512 * j:512 * j + 512],
                            opsum[obase:obase + 64, :],
                            rec[obase:obase + 64, :])

                    stiles, ptts, opsums = {}, {}, {}
                    pc_queue = list(range(8)) if j > 0 else []
                    for g in range(G + 2):
                        # PE filler with no ACT dependency first
                        if pc_queue and g >= 6:
                            emit_phase_c_group(j - 1, pc_queue.pop(0))
                        if 2 <= g:
                            tr = trips[g - 2]
                            h = tr[1]
                            is_first_trip_of_h = (tr[0] == "diagA" and j == 0) or \
                                (tr[0] == "full" and tr[2][0] == 0)
                            if is_first_trip_of_h:
                                opsums[h] = psB.tile([128, 512], F32, name="opsum",
                                                     tag="acc", bufs=2)
                            emit_pv(tr, ptts.pop(g - 2), opsums[h], is_first_trip_of_h)
                            if tr[0] == "diagB":
                                op_done = opsums.pop(h)
                                if debug and j == 0 and h == 0:
                                    dbg_op_src = op_done
                                emit_norm(h, op_done)
                        if g < G:
                            stiles[g] = emit_scores(trips[g])
                            if debug and j == 0 and g == 0:
                                dbg_sb = pb.tile([128, 1024], F32R, name="dbgsb",
                                                 tag="dbgsb")
                                nc.vector.tensor_copy(dbg_sb[:, :], stiles[g][:, :])
                                nc.sync.dma_start(out=st_d[:, :], in_=dbg_sb[:, :])
                        if 1 <= g <= G:
                            ptts[g - 1] = emit_exp(trips[g - 1], stiles.pop(g - 1))
                            if debug and j == 0 and g == 1:
                                nc.sync.dma_start(out=pt_d[:, :], in_=ptts[g - 1][:, :])
                            if debug and j == 0 and g == 4:
                                dbg_op = pb.tile([128, 512], F32R, name="dbgop",
                                                 tag="dbgop")
                                nc.vector.tensor_copy(dbg_op[:, :], dbg_op_src[:, :])
                                nc.sync.dma_start(out=op_d[:, :], in_=dbg_op[:, :])
                    # drain any leftover phase C groups of j-1
                    while pc_queue:
                        emit_phase_c_group(j - 1, pc_queue.pop(0))

                # final out-proj for j=3
                for e in range(8):
                    emit_phase_c_group(NJ - 1, e)

                if debug:
                    for h in range(HPC):
                        nc.sync.dma_start(out=qt_d[h, :, :], in_=qtz[h][:, :])
                    for p in range(2):
                        nc.sync.dma_start(out=kt_d[p, :, :], in_=kt[p][:, :])
                        nc.sync.dma_start(out=ot_d[p, :, :], in_=ot2[p][:, :])
                    nc.sync.dma_start(out=v_d[:, :, :, :], in_=v2[:, :, :, :])

    nc.compile()
    return nc


def _run(inputs, trace=False, debug=False):
    global _NC
    if _NC is None:
        _NC = _build(debug=debug)
    q = np.asarray(inputs["q"], dtype=np.float32)
    k = np.asarray(inputs["k"], dtype=np.float32)
    v = np.asarray(inputs["v"], dtype=np.float32)
    w_query = np.asarray(inputs["w_query"], dtype=np.float32)
    w_key = np.asarray(inputs["w_key"], dtype=np.float32)
    w_value = np.asarray(inputs["w_value"], dtype=np.float32)
    w_0 = np.asarray(inputs["w_0"], dtype=np.float32)

    tri = np.triu(np.ones((128, 128), dtype=np.float32))
    cmt = np.ascontiguousarray(np.tile(tri, (1, 4)))

    xq_b = [np.ascontiguousarray(q[b].T).astype(np.float16) for b in range(B)]
    xk_b = [np.ascontiguousarray(k[b].T).astype(np.float16) for b in range(B)]
    xv_b = [np.ascontiguousarray(v[b].T).astype(np.float16) for b in range(B)]

    in_maps = []
    for c in range(8):
        b, g = c // 4, c % 4
        sl = slice(256 * g, 256 * g + 256)
        in_maps.append({
            "xq": xq_b[b], "xk": xk_b[b], "xv": xv_b[b],
            "wq": np.ascontiguousarray(w_query[sl, :].T).astype(np.float16),
            "wk": np.ascontiguousarray(w_key[sl, :].T).astype(np.float16),
            "wv": np.ascontiguousarray(w_value[sl, :].T).astype(np.float16),
            "w0": np.ascontiguousarray(w_0[:, sl].T).astype(np.float16),
            "cmt": cmt,
        })

    res = run_bass_kernel_spmd(_NC, in_maps, core_ids=list(range(8)), trace=trace)
    y = np.empty((B, S, D), dtype=np.float32)
    for b in range(B):
        acc = res.results[4 * b]["yt"].astype(np.float32)
        for g in range(1, 4):
            acc += res.results[4 * b + g]["yt"].astype(np.float32)
        y[b] = acc.T
    if debug:
        return y, getattr(res, "exec_time_ns", None), res
    return y, getattr(res, "exec_time_ns", None)


def kernel(**inputs):
    return _run(inputs, trace=False)[0]


# revision 39
# speedup vs baseline: 1.0500x; 1.0100x over previous
import sys

sys.path.insert(0, "/opt/trn_rl_repo")
import numpy as np
import concourse.bacc as bacc
import concourse.mybir as mybir
import concourse.tile as tile
from concourse.bass_utils import run_bass_kernel_spmd

F32R = mybir.dt.float32r
F32 = mybir.dt.float32
FP16 = mybir.dt.float16
AF = mybir.ActivationFunctionType

B, S, D, H, DV = 2, 2048, 1024, 16, 64
NKT = 8     # k-tiles of 128 over D
NJ = 4      # query chunks of 512
NB = 16     # key blocks of 128
HPC = 4     # heads per core

_NC = None


def _build(debug=False):
    nc = bacc.Bacc(target_bir_lowering=False)
    xq = nc.dram_tensor("xq", [D, S], FP16, kind="ExternalInput")
    xk = nc.dram_tensor("xk", [D, S], FP16, kind="ExternalInput")
    xv = nc.dram_tensor("xv", [D, S], FP16, kind="ExternalInput")
    wq = nc.dram_tensor("wq", [D, 256], FP16, kind="ExternalInput")
    wk = nc.dram_tensor("wk", [D, 256], FP16, kind="ExternalInput")
    wv = nc.dram_tensor("wv", [D, 256], FP16, kind="ExternalInput")
    w0 = nc.dram_tensor("w0", [256, D], FP16, kind="ExternalInput")
    cmt = nc.dram_tensor("cmt", [128, 512], F32R, kind="ExternalInput")
    yt = nc.dram_tensor("yt", [D, S], FP16, kind="ExternalOutput")
    if debug:
        qt_d = nc.dram_tensor("qt_d", [4, 128, S], FP16, kind="ExternalOutput")
        kt_d = nc.dram_tensor("kt_d", [2, 128, S], FP16, kind="ExternalOutput")
        v_d = nc.dram_tensor("v_d", [128, NB, HPC, 128], F32R, kind="ExternalOutput")
        ot_d = nc.dram_tensor("ot_d", [2, 128, S], FP16, kind="ExternalOutput")
        st_d = nc.dram_tensor("st_d", [128, 1024], F32R, kind="ExternalOutput")
        pt_d = nc.dram_tensor("pt_d", [128, 1024], F32R, kind="ExternalOutput")
        op_d = nc.dram_tensor("op_d", [128, 512], F32R, kind="ExternalOutput")

    with tile.TileContext(nc) as tc:
        with tc.tile_pool(name="pp", bufs=1) as pp:
            # Per-head Q with the other head's 64 rows zeroed: scores can then
            # use the full dense 128-row kt block as stationary (full PE
            # array) -- the zero rows kill the other head's contribution.
            qtz = [pp.tile([128, S], FP16, name=f"qtz{h}", tag=f"qtz{h}")
                   for h in range(HPC)]
            kt = [pp.tile([128, S], FP16, name=f"kt{p}", tag=f"kt{p}") for p in range(2)]
            # V padded to 128 cols, all non-V columns = ones. Even heads keep
            # V in cols 0:64 (numerators -> out rows 0:64, den at row 64);
            # odd heads keep V in cols 64:128 (numerators -> out rows 64:128,
            # den read from row 32). This keeps every normalize step
            # partition-aligned and lets the out-projection consume a
            # pair-stacked [128, S] activation with a full 128-row stationary.
            v2 = pp.tile([128, NB, HPC, 128], F32R, name="v2", tag="v2")
            # bcsel rows 0 and 64: all-ones [1,128] stationaries that
            # broadcast the den row (even heads: opsum row 64; odd heads:
            # opsum row 0) onto all 128 partitions of bcps.
            bcsel = pp.tile([128, 128], F32R, name="bcsel", tag="bcsel")
            w0p = [pp.tile([128, D], FP16, name=f"w0p{p}", tag=f"w0p{p}") for p in range(2)]
            ot2 = [pp.tile([128, S], FP16, name=f"ot2{p}", tag=f"ot2{p}") for p in range(2)]
            cm_sb = pp.tile([128, 512], F32R, name="cmsb", tag="cmsb")

            # ones / zero init (stays valid for the whole kernel)
            ones_stage = pp.tile([128, 512], F32, name="ones_stage", tag="ones_stage")
            nc.vector.memset(ones_stage[:, :], 1.0)
            for i in range(NB):
                nc.vector.tensor_copy(v2[:, i, :, :], ones_stage[:, :])
            nc.vector.tensor_copy(bcsel[64:65, :], ones_stage[64:65, 0:128])
            nc.vector.tensor_copy(bcsel[0:1, :], ones_stage[0:1, 0:128])
            for h in range(HPC):
                dead = 64 * (1 - (h % 2))
                nc.vector.memset(qtz[h][dead:dead + 64, :], 0.0)

            # ---- Phase A: projections ----
            with tc.tile_pool(name="wts", bufs=1) as wts, \
                 tc.tile_pool(name="xin", bufs=1) as xin, \
                 tc.tile_pool(name="psA", bufs=8, space="PSUM") as psA:
                # DMA order: each weight tensor lands just before the x
                # tiles that feed its projection; x tiles round-robin across
                # the two hw DMA queues in PE consumption order.
                wv_t, wq_t, wk_t = [], [], []
                for lst, dram, tag in ((wv_t, wv, "wv"), (wq_t, wq, "wq"),
                                       (wk_t, wk, "wk")):
                    for k in range(NKT):
                        t = wts.tile([128, 256], FP16, name=f"{tag}{k}", tag=f"{tag}{k}")
                        lst.append(t)
                xv_t, xq_t, xk_t = [], [], []
                for lst, dram, tag in ((xv_t, xv, "xv"), (xq_t, xq, "xq"),
                                       (xk_t, xk, "xk")):
                    for k in range(NKT):
                        t = xin.tile([128, S], FP16, name=f"{tag}{k}", tag=tag,
                                     bufs=8)
                        lst.append(t)
                for wlst, wdram, xlst, xdram in ((wv_t, wv, xv_t, xv),
                                                 (wq_t, wq, xq_t, xq),
                                                 (wk_t, wk, xk_t, xk)):
                    for k in range(NKT):
                        nc.scalar.dma_start(out=wlst[k][:, :],
                                            in_=wdram[128 * k:128 * k + 128, :])
                    for k in range(NKT):
                        eng = nc.sync if k % 2 == 0 else nc.scalar
                        eng.dma_start(out=xlst[k][:, :],
                                      in_=xdram[128 * k:128 * k + 128, :])
                for p in range(2):
                    nc.sync.dma_start(out=w0p[p][:, :], in_=w0[128 * p:128 * p + 128, :])
                nc.sync.dma_start(out=cm_sb[:, :], in_=cmt[:, :])

                # V projection: 2 waves x 8 st-groups, kt-outer within a wave
                for w in range(2):
                    vps = [psA.tile([128, 4, 64], F32, name=f"vps{w}{g}", tag="pj",
                                    bufs=8)
                           for g in range(8)]
                    for k in range(NKT):
                        for g in range(8):
                            st = 8 * w + g
                            nc.tensor.matmul(
                                vps[g][:, :, :],
                                xv_t[k][:, 128 * st:128 * st + 128],
                                wv_t[k][:, :],
                                start=(k == 0), stop=(k == NKT - 1))
                    for g in range(8):
                        nc.vector.tensor_copy(v2[:, 8 * w + g, 0:4:2, 0:64],
                                              vps[g][:, 0:4:2, :])
                        nc.vector.tensor_copy(v2[:, 8 * w + g, 1:4:2, 64:128],
                                              vps[g][:, 1:4:2, :])

                # Q/K projections: N=256 matmuls, stationary stable across 8 cols
                for which, wt, xt in (("q", wq_t, xq_t), ("k", wk_t, xk_t)):
                    qps = [psA.tile([128, 512], F32, name=f"pj{i}", tag="pj", bufs=8)
                           for i in range(8)]
                    for k in range(NKT):
                        for p in range(2):
                            for c in range(8):
                                jj, half = c // 2, c % 2
                                nc.tensor.matmul(
                                    qps[4 * p + jj][:, 256 * half:256 * half + 256],
                                    wt[k][:, 128 * p:128 * p + 128],
                                    xt[k][:, 256 * c:256 * c + 256],
                                    start=(k == 0 and half == 0),
                                    stop=(k == NKT - 1 and half == 1))
                    for p in range(2):
                        for jj in range(4):
                            sl = slice(512 * jj, 512 * jj + 512)
                            if which == "q":
                                nc.vector.tensor_copy(qtz[2 * p][0:64, sl],
                                                      qps[4 * p + jj][0:64, :])
                                nc.vector.tensor_copy(qtz[2 * p + 1][64:128, sl],
                                                      qps[4 * p + jj][64:128, :])
                            else:
                                nc.vector.tensor_copy(kt[p][:, sl],
                                                      qps[4 * p + jj][:, :])

            # ---- Phase B/C: flat software-pipelined attention + out-proj ----
            with tc.tile_pool(name="pb", bufs=1) as pb, \
                 tc.tile_pool(name="psB", bufs=1, space="PSUM") as psB:

                def emit_phase_c_group(jp, e):
                    yps = psB.tile([128, 512], F32, name="yps", tag="yps", bufs=2)
                    for p in range(2):
                        for half in range(2):
                            nc.tensor.matmul(
                                yps[:, 256 * half:256 * half + 256],
                                w0p[p][:, 128 * e:128 * e + 128],
                                ot2[p][:, 512 * jp + 256 * half:512 * jp + 256 * half + 256],
                                start=(p == 0 and half == 0),
                                stop=(p == 1 and half == 1))
                    ysb = pb.tile([128, 512], FP16, name="ysb", tag="ysb", bufs=3)
                    nc.vector.tensor_copy(ysb[:, :], yps[:, :])
                    nc.sync.dma_start(out=yt[128 * e:128 * e + 128, 512 * jp:512 * jp + 512],
                                      in_=ysb[:, :])

                # HAM warm-up burst: dependency-free full-array matmuls to
                # carry K=8/8 across the phase A->B transition bubble.
                wps = psB.tile([128, 512], F32, name="wps", tag="yps", bufs=2)
                for wi in range(24):
                    nc.tensor.matmul(
                        wps[:, 256 * (wi % 2):256 * (wi % 2) + 256],
                        kt[wi % 2][:, 0:128],
                        qtz[wi % 4][:, 0:256],
                        start=(wi < 2), stop=(wi >= 22))

                for j in range(NJ):
                    trips = []
                    for h in range(HPC):
                        for t in range(2 * j):
                            trips.append(("full", h, [2 * t, 2 * t + 1]))
                        trips.append(("diagA", h, None))
                        trips.append(("diagB", h, None))
                    G = len(trips)

                    def sc_mm(stile, col, width, blk, qoff, pair, h, first, last):
                        nc.tensor.matmul(
                            stile[:, col:col + width],
                            kt[pair][:, 128 * blk:128 * blk + 128],
                            qtz[h][:, qoff:qoff + width],
                            start=first, stop=last)

                    def emit_scores(tr):
                        kind, h, blks = tr
                        pair = h // 2
                        qb = 512 * j
                        stile = psB.tile([128, 1024], F32, name="stile", tag="stile", bufs=2)
                        if kind == "full":
                            for n, blk in enumerate(blks):
                                for half in range(2):
                                    sc_mm(stile, 512 * n + 256 * half, 256, blk,
                                          qb + 256 * half, pair, h,
                                          half == 0, half == 1)
                        elif kind == "diagA":
                            for dd in range(4):
                                sc_mm(stile, 128 * dd, 128, 4 * j + dd,
                                      qb + 128 * dd, pair, h, dd == 0, dd == 3)
                            sc_mm(stile, 512, 256, 4 * j, qb + 128, pair, h, True, False)
                            sc_mm(stile, 768, 128, 4 * j, qb + 384, pair, h, False, True)
                        else:  # diagB
                            sc_mm(stile, 0, 256, 4 * j + 1, qb + 256, pair, h, True, False)
                            sc_mm(stile, 256, 128, 4 * j + 2, qb + 384, pair, h, False, True)
                        return stile

                    def emit_exp(tr, stile):
                        kind, h, blks = tr
                        w = {"full": 1024, "diagA": 896, "diagB": 384}[kind]
                        ptt = pb.tile([128, 1024], F32R, name="ptt", tag="ptt", bufs=3)
                        nc.scalar.activation(ptt[:, 0:w], stile[:, 0:w], AF.Exp)
                        if kind == "diagA":
                            nc.vector.tensor_mul(ptt[:, 0:512], ptt[:, 0:512], cm_sb[:, :])
                        return ptt

                    def emit_pv(tr, ptt, opsum, first):
                        kind, h, blks = tr

                        def pv(col, width, blk, pcol, fb, lb=False):
                            nc.tensor.matmul(
                                opsum[:, col:col + width],
                                v2[:, blk, h, :],
                                ptt[:, pcol:pcol + width],
                                start=fb, stop=lb)

                        if kind == "full":
                            for n, blk in enumerate(blks):
                                for half in range(2):
                                    pv(256 * half, 256, blk, 512 * n + 256 * half,
                                       first and n == 0 and half == 0)
                        elif kind == "diagA":
                            for dd in range(4):
                                pv(128 * dd, 128, 4 * j + dd, 128 * dd, first and dd == 0)
                            pv(128, 256, 4 * j, 512, False)
                            pv(384, 128, 4 * j, 768, False)
                        else:
                            pv(256, 256, 4 * j + 1, 0, False)
                            pv(384, 128, 4 * j + 2, 256, False, lb=True)

                    def emit_norm(h, opsum):
                        # den row -> SBUF, PE-broadcast onto the head's own
                        # partition half, reciprocal, normalize.  Even heads:
                        # num rows 0:64, den row 64.  Odd heads: num rows
                        # 64:128, den row 0 (every ones column repeats den).
                        drow = 64 if h % 2 == 0 else 0
                        obase = 0 if h % 2 == 0 else 64
                        pair = h // 2
                        den = pb.tile([128, 512], F32R, name="den", tag="den", bufs=2)
                        nc.vector.tensor_copy(den[drow:drow + 1, :],
                                              opsum[drow:drow + 1, :])
                        bcps = psB.tile([128, 512], F32, name="bcps", tag="yps",
                                        bufs=2)
                        for half in range(2):
                            nc.tensor.matmul(
                                bcps[:, 256 * half:256 * half + 256],
                                bcsel[drow:drow + 1, :],
                                den[drow:drow + 1, 256 * half:256 * half + 256],
                                start=(half == 0), stop=(half == 1))
                        rec = pb.tile([128, 512], F32, name="rec", tag="rec", bufs=2)
                        nc.vector.reciprocal_approx_fast(rec[:, :], bcps[:, :])
                        nc.vector.tensor_mul(
                            ot2[pair][obase:obase + 64, 512 * j:512 * j + 512],
                            opsum[obase:obase + 64, :],
                            rec[obase:obase + 64, :])

                    stiles, ptts, opsums = {}, {}, {}
                    pc_queue = list(range(8)) if j > 0 else []
                    for g in range(G + 2):
                        # PE filler with no ACT dependency first
                        if pc_queue and g >= 6:
                            emit_phase_c_group(j - 1, pc_queue.pop(0))
                        if 2 <= g:
                            tr = trips[g - 2]
                            h = tr[1]
                            is_first_trip_of_h = (tr[0] == "diagA" and j == 0) or \
                                (tr[0] == "full" and tr[2][0] == 0)
                            if is_first_trip_of_h:
                                opsums[h] = psB.tile([128, 512], F32, name="opsum",
                                                     tag="acc", bufs=2)
                            emit_pv(tr, ptts.pop(g - 2), opsums[h], is_first_trip_of_h)
                            if tr[0] == "diagB":
                                op_done = opsums.pop(h)
                                if debug and j == 0 and h == 0:
                                    dbg_op_src = op_done
                                emit_norm(h, op_done)
                        if g < G:
                            stiles[g] = emit_scores(trips[g])
                            if debug and j == 0 and g == 0:
                                dbg_sb = pb.tile([128, 1024], F32R, name="dbgsb",
                                                 tag="dbgsb")
                                nc.vector.tensor_copy(dbg_sb[:, :], stiles[g][:, :])
                                nc.sync.dma_start(out=st_d[:, :], in_=dbg_sb[:, :])
                        if 1 <= g <= G:
                            ptts[g - 1] = emit_exp(trips[g - 1], stiles.pop(g - 1))
                            if debug and j == 0 and g == 1:
                                nc.sync.dma_start(out=pt_d[:, :], in_=ptts[g - 1][:, :])
                            if debug and j == 0 and g == 4:
                                dbg_op = pb.tile([128, 512], F32R, name="dbgop",
                                                 tag="dbgop")
                                nc.vector.tensor_copy(dbg_op[:, :], dbg_op_src[:, :])
                                nc.sync.dma_start(out=op_d[:, :], in_=dbg_op[:, :])
                    # drain any leftover phase C groups of j-1
                    while pc_queue:
                        emit_phase_c_group(j - 1, pc_queue.pop(0))

                # final out-proj for j=3
                for e in range(8):
                    emit_phase_c_group(NJ - 1, e)

                if debug:
                    for h in range(HPC):
                        nc.sync.dma_start(out=qt_d[h, :, :], in_=qtz[h][:, :])
                    for p in range(2):
                        nc.sync.dma_start(out=kt_d[p, :, :], in_=kt[p][:, :])
                        nc.sync.dma_start(out=ot_d[p, :, :], in_=ot2[p][:, :])
                    nc.sync.dma_start(out=v_d[:, :, :, :], in_=v2[:, :, :, :])

    nc.compile()
    return nc


def _run(inputs, trace=False, debug=False):
    global _NC
    if _NC is None:
        _NC = _build(debug=debug)
    q = np.asarray(inputs["q"], dtype=np.float32)
    k = np.asarray(inputs["k"], dtype=np.float32)
    v = np.asarray(inputs["v"], dtype=np.float32)
    w_query = np.asarray(inputs["w_query"], dtype=np.float32)
    w_key = np.asarray(inputs["w_key"], dtype=np.float32)
    w_value = np.asarray(inputs["w_value"], dtype=np.float32)
    w_0 = np.asarray(inputs["w_0"], dtype=np.float32)

    tri = np.triu(np.ones((128, 128), dtype=np.float32))
    cmt = np.ascontiguousarray(np.tile(tri, (1, 4)))

    xq_b = [np.ascontiguousarray(q[b].T).astype(np.float16) for b in range(B)]
    xk_b = [np.ascontiguousarray(k[b].T).astype(np.float16) for b in range(B)]
    xv_b = [np.ascontiguousarray(v[b].T).astype(np.float16) for b in range(B)]

    in_maps = []
    for c in range(8):
        b, g = c // 4, c % 4
        sl = slice(256 * g, 256 * g + 256)
        in_maps.append({
            "xq": xq_b[b], "xk": xk_b[b], "xv": xv_b[b],
            "wq": np.ascontiguousarray(w_query[sl, :].T).astype(np.float16),
            "wk": np.ascontiguousarray(w_key[sl, :].T).astype(np.float16),
            "wv": np.ascontiguousarray(w_value[sl, :].T).astype(np.float16),
            "w0": np.ascontiguousarray(w_0[:, sl].T).astype(np.float16),
            "cmt": cmt,
        })

    res = run_bass_kernel_spmd(_NC, in_maps, core_ids=list(range(8)), trace=trace)
    y = np.empty((B, S, D), dtype=np.float32)
    for b in range(B):
        acc = res.results[4 * b]["yt"].astype(np.float32)
        for g in range(1, 4):
            acc += res.results[4 * b + g]["yt"].astype(np.float32)
        y[b] = acc.T
    if debug:
        return y, getattr(res, "exec_time_ns", None), res
    return y, getattr(res, "exec_time_ns", None)


def kernel(**inputs):
    return _run(inputs, trace=False)[0]
